# revision 5
# baseline (speedup 1.0000x reference)
"""Trainium2 Bass kernel for nn_CMLITargetLoss — fp8 DoubleRow version.

Reference semantics (B=64, L=197, D=768):
    sim[b,i,t,p] = text[b,t,:] . image[i,p,:]      (masked where padding_masks[b,p])
    token2patch  = argmax over p of sim[:, :, 1:, 1:]
    only the diagonal (b == i) of token2patch is used:
        aligned[b,t] = image[b, 1 + token2patch[b,b,t]]
        kd_token = mean((text[:,1:] - aligned)^2)
    kd_cls  = mean((image[:,0] - target[:,0])^2)
    loss = kd_cls + kd_token

Algebraic reduction per sample b (tokens/patches t,p in 1..196):
    S[t,p]   = text_t . image_p  over d<767, with hidden channel 767
               repurposed as the mask lane: textT[767,:] = -240 and
               imageT[767,p] = 240*mask_p, so masked patches carry a -57600
               bias and can never win the argmax (any mask pattern). The
               dropped d=767 term only perturbs S by ~N(0,1) (vs max-gaps
               of O(10)); all exact loss terms use the full 768 dims on
               the host.
    M[t]     = max_p S[t,p]                  == text_t . aligned_t
    E[t,p]   = sign(M[t] - S[t,p]) in {0 (at argmax), 1}
    cntE[p]  = sum_t E[t,p] = T - cnt[p]
    gE       = sum_p cntE[p]*in2[p],  in2[p] = ||image_p||^2 (host, fp32)
    sum_t ||text_t - aligned_t||^2
        = sum||text||^2 - 2*sum M + (T*sum_p in2[p] - gE)

The device computes only sum(M) and sum(gE) (the quadratic-cost part that
needs the full T x T similarity); sum||text||^2, in2, and the CLS loss are
linear-time reductions of the raw inputs folded in on the host.

Performance design (per core: 8 samples, ~2.3 MB fp8 input = the memory
roofline at ~360 GB/s; CoreSim-predicted 12.7 us vs 35.4 us for the staged
bf16 baseline):
  - inputs shipped as fp8e4; S matmuls use MatmulPerfMode.DoubleRow (K=256
    per instruction at 0.5 cycles/row = 4x bf16 PE throughput). Dual-fp8
    Ldweights requires a 16B-aligned slab stride (walrus
    s3_lw_dual_fp8_restrictions), so the textT (stationary) blocks pad
    tokens 196->208; the imageT (moving) blocks are unrestricted.
  - PSUM rows 64:128 of the 68-token chunk are pre-written with a
    zeros x zeros matmul (no DMA dependency): the whole [128, 2, T] block is
    defined, ONE DVE reduce covers both chunks, and the fake rows contribute
    exactly 0 to both sum(M) and cnt (sign(0-0) = 0).
  - E = sign(M - S) built on the Act engine (bias=+M per partition,
    scale=-1), fp8 out; a dummy Sign at t=0 pre-loads the act table. Three
    samples' chunk-1 indicators run on the DVE instead (is_equal, taken
    with sel=-1 and k_s=68 in the host decode) to balance the Act stream.
  - per-sample cnt matmuls accumulate into ONE [8, T] PSUM tile via a
    selector lhsT (column s = +-1 per slab) so a single DVE
    scalar_tensor_tensor computes all samples' gE partial sums at the end.
  - ti[0] + consts ride the Act sequencer's shorter DMA chain; the raw
    [128, 18] result block ships in three overlapping DMAs so only the
    8-value gE column waits for the final DVE op.
"""

import os
import sys

import numpy as np

for _p in ("/opt/trn_rl_repo", "/root/.axon_site/_ro/trn_rl_repo"):
    if _p not in sys.path and os.path.isdir(_p):
        sys.path.insert(0, _p)

import ml_dtypes

import concourse.bass as bass
import concourse.tile as tile
from concourse import mybir
from concourse.bass_utils import run_bass_kernel_spmd

F32 = mybir.dt.float32
FP8 = mybir.dt.float8e4
NP_FP8 = ml_dtypes.float8_e4m3
ALU = mybir.AluOpType
AX = mybir.AxisListType
ACTF = mybir.ActivationFunctionType
DR = mybir.MatmulPerfMode.DoubleRow

B, L, D = 64, 197, 768
NCORES = 8
SPC = B // NCORES          # samples per core
T = L - 1                  # 196 tokens / patches after dropping CLS
TP = 208                   # T padded: dual-fp8 Ldweights needs 16B-aligned
                           # slab stride (walrus s3_lw_dual_fp8_restrictions)
KC = D // 256              # 3 DoubleRow contraction chunks of 2x128
NEG = -240.0               # e4m3 max normal; applied twice -> -480 mask bias
TCHUNKS = ((0, 128), (128, T - 128))   # token-dim partition chunks: 128 + 68
DVE_IND = (1, 5, 7)        # samples whose chunk-1 indicator runs on the DVE


def build_nc(split_waits: bool = True) -> bass.Bass:
    nc = bass.Bass()

    # per-sample packed text+image, fp8, exact SBUF layout (one DMA each).
    # Per partition: [textT [KC, 2, TP] (tokens padded 196->208: dual-fp8
    # Ldweights needs a 16B-aligned slab stride) | imageT [KC, 2, T]
    # (rhs side is unrestricted)], with d = c*256 + i*128 + p
    ti = nc.declare_dram_parameter(
        "ti", [SPC, 128, KC * 2 * (TP + T)], FP8, isOutput=False
    )
    # selector lhsT for the cnt matmul: sel[p, s, i, m] = (m == s)
    sel = nc.declare_dram_parameter("sel", [128, SPC, 2, 16], FP8, isOutput=False)
    # host-computed ||image_p||^2, [s, p] fp32
    in2 = nc.declare_dram_parameter("in2", [SPC, T], F32, isOutput=False)
    # raw results [128, 9, 2, 8]: [:, s, j, 0] = chunk-j M of sample s;
    # [:8, 8, 0, 0] = gE partials. 32B-wide slots keep concurrent writers
    # and readers in separate dependency-tracking granules.
    out = nc.declare_dram_parameter(
        "out", [128, (SPC + 1) * 2 * 8], F32, isOutput=True
    )

    with tile.TileContext(nc) as tc:
        _emit(nc, tc, ti, sel, in2, out)
    if split_waits:  # CoreSim can't execute the injected NoOps; HW needs them
        _split_multiwaits(nc)
    return nc


# The walrus build in this container only supports a single semaphore-wait
# command per instruction. Tile freely attaches several. Hoist all but one
# wait of every instruction onto same-engine NoOps placed directly before it.
def _split_multiwaits(nc):
    CARRIERS = ("InstNoOp", "InstEventSemaphore")
    for bb in nc.main_func.blocks:
        new = []
        for ins in bb.instructions:
            si = ins.sync_info
            if (
                si is not None
                and si.on_wait
                and len(si.on_wait) > 1
                and type(ins).__name__ not in CARRIERS
            ):
                waits = list(si.on_wait)
                for w in waits[:-1]:
                    nop = mybir.InstNoOp(
                        name=nc.get_next_instruction_name(),
                        engine=ins.engine,
                        ins=[],
                        outs=[],
                        sync_info=mybir.SyncInfo(on_wait=[w], on_update=[]),
                    )
                    new.append(nop)
                ins.sync_info = mybir.SyncInfo(
                    on_wait=[waits[-1]], on_update=list(si.on_update)
                )
            new.append(ins)
        bb.instructions[:] = new


def _emit(nc, tc, ti, sel, in2, out):
    with (
        tc.tile_pool(name="big", bufs=1) as big,
        tc.tile_pool(name="small", bufs=1) as small,
        tc.tile_pool(name="ework", bufs=3) as ework,
        tc.tile_pool(name="psS", bufs=6, space="PSUM") as psS,
        tc.tile_pool(name="pscnt", bufs=1, space="PSUM") as pscnt,
    ):
        # dummy Sign pre-loads the Act engine's function table while the
        # first DMA is still in its descriptor chain; its tiny input memset
        # goes first so the table load starts as early as possible
        warm = small.tile([1, 2], FP8, tag="warm")
        nc.vector.memset(warm, 0.0)
        nc.scalar.activation(out=warm, in_=warm, func=ACTF.Sign, scale=-1.0)
        # zero scratch (memset, no DMA): operands of the fake-rows matmul
        zrow = small.tile([1, 2, T], FP8, tag="zrow")
        nc.vector.memset(zrow, 0.0)
        # results tile [128, 9, 2, 8]; shipped out raw, host does the sums
        Ms = small.tile([128, SPC + 1, 2, 8], F32, tag="Ms")
        nc.vector.memset(Ms, 0.0)
        # ---- DMAs. ti[0] + consts ride the Act sequencer (its DMA chain is
        # ~0.4us shorter than SP's, and the const transfers are tiny); bulk
        # samples 1..7 stream on SP concurrently.
        W8 = KC * 2 * (TP + T)
        TXW = KC * 2 * TP
        tis = [None] * SPC
        tis[0] = big.tile([128, W8], FP8, tag="ti0", name="ti0")
        nc.scalar.dma_start(out=tis[0], in_=ti[0])
        sel_sb = small.tile([128, SPC, 2, 16], FP8, tag="sel")
        nc.scalar.dma_start(out=sel_sb, in_=sel[:, :, :, :])
        in2_sb = small.tile([SPC, T], F32, tag="in2")
        nc.scalar.dma_start(out=in2_sb, in_=in2[:, :])

        for s in range(1, SPC):
            tis[s] = big.tile([128, W8], FP8, tag=f"ti{s}", name=f"ti{s}")
            nc.sync.dma_start(out=tis[s], in_=ti[s])

        gscr = small.tile([SPC, T], F32, tag="gscr")

        etiles = [
            ework.tile([128, 2, 224], FP8, tag="E", name=f"E{i}") for i in range(3)
        ]
        # the last sample gets its own single-slab indicator tiles and its
        # own M-column tiles so no two tail ops ever share a tile (the
        # dependency tracker serializes reads/writes at tile granularity)
        e7a = small.tile([128, 224], FP8, tag="e7a")
        e7b = small.tile([128, 224], FP8, tag="e7b")
        m7a = small.tile([128, 8], F32, tag="m7a")
        nc.vector.memset(m7a, 0.0)
        m7b = small.tile([128, 8], F32, tag="m7b")
        nc.vector.memset(m7b, 0.0)
        mg = small.tile([SPC, 8], F32, tag="mg")
        nc.vector.memset(mg, 0.0)

        # single [8, T] PSUM accumulator for all samples' cntE rows
        ps_cnt = pscnt.tile([SPC, T], F32, tag="cnt")

        # ---- per-sample pipeline (cnt matmul of s-1 issued after S of s,
        # so the PE never stalls on the Act engine's Sign pass) ----
        pending_cnt = None  # (E-tile, sample)
        ncnt = 0
        for s in range(SPC):
            tt = tis[s][:, :TXW].rearrange(
                "p (c i x) -> p c i x", c=KC, i=2, x=TP
            )                      # [128, KC, 2, TP] textT  (fp8)
            it = tis[s][:, TXW:].rearrange(
                "p (c i x) -> p c i x", c=KC, i=2, x=T
            )                      # [128, KC, 2, T] imageT (fp8)

            ps = psS.tile([128, 2, T], F32, tag="psS")
            # fake rows 64:128 of chunk 1: zeros x zeros -> S=0 (rows 64:68
            # overwritten by the real chunk-1 group). M=0 and E=sign(0)=0
            # there, so they pollute nothing, and the whole [128, 2, T]
            # block is defined for the single reduce below.
            nc.tensor.matmul(
                ps[64:128, 1, :], lhsT=zrow[:, 0, 0:64], rhs=zrow[:, 0, :],
                start=True, stop=True, skip_group_check=True,
            )
            for j, (t0, mj) in enumerate(TCHUNKS):
                for c in range(KC):
                    nc.tensor.matmul(
                        ps[:mj, j, :],
                        lhsT=tt[:, c, :, t0 : t0 + mj], rhs=it[:, c],
                        start=(c == 0), stop=(c == KC - 1), perf_mode=DR,
                    )

            # E = sign(M - S) in {0 at argmax, 1 elsewhere}. The first and
            # last samples split the reduce per chunk and interleave the
            # chunk-0 indicator between the two reduces, so it only depends
            # on its own M column (emitting it after both reduces creates a
            # false tile-granular wait on the chunk-1 reduce). Two samples'
            # chunk-1 indicator runs on the DVE as is_equal (sel slab-1 is
            # -1, host uses k_s = 68); the last sample writes dedicated
            # single-slab tiles so Act and DVE indicators fully overlap.
            E = etiles[s % 3]
            if s == 0 or s == SPC - 1:
                last = s == SPC - 1
                o0 = e7a[:, :T] if last else E[:, 0, :T]
                o1 = e7b[:, :T] if last else E[:, 1, :T]
                c0 = m7a[:, 0:1] if last else Ms[:, s, 0, 0:1]
                c1 = m7b[:, 0:1] if last else Ms[:, s, 1, 0:1]
                nc.vector.tensor_reduce(
                    out=c0, in_=ps[:, 0, :], axis=AX.X, op=ALU.max
                )
                nc.scalar.activation(
                    out=o0, in_=ps[:, 0, :], func=ACTF.Sign,
                    bias=c0, scale=-1.0,
                )
                nc.vector.tensor_reduce(
                    out=c1, in_=ps[:, 1, :], axis=AX.X, op=ALU.max
                )
                if s in DVE_IND:
                    nc.vector.tensor_scalar(
                        o1, ps[:, 1, :], c1, None, ALU.is_equal
                    )
                else:
                    nc.scalar.activation(
                        out=o1, in_=ps[:, 1, :], func=ACTF.Sign,
                        bias=c1, scale=-1.0,
                    )
            else:
                nc.vector.tensor_reduce(
                    out=Ms[:, s, :, 0], in_=ps[:, :, :], axis=AX.X, op=ALU.max
                )
                nc.scalar.activation(
                    out=E[:, 0, :T], in_=ps[:, 0, :], func=ACTF.Sign,
                    bias=Ms[:, s, 0, 0:1], scale=-1.0,
                )
                if s in DVE_IND:
                    nc.vector.tensor_scalar(
                        E[:, 1, :T], ps[:, 1, :], Ms[:, s, 1, 0:1], None,
                        ALU.is_equal,
                    )
                else:
                    nc.scalar.activation(
                        out=E[:, 1, :T], in_=ps[:, 1, :], func=ACTF.Sign,
                        bias=Ms[:, s, 1, 0:1], scale=-1.0,
                    )

            if pending_cnt is not None:
                pE, pS = pending_cnt
                nc.tensor.matmul(
                    ps_cnt, lhsT=sel_sb[:, pS, :, :SPC], rhs=pE[:, :, :T],
                    start=(ncnt == 0), stop=False, perf_mode=DR,
                    skip_group_check=True,
                )
                ncnt += 1
            pending_cnt = (E, s)

        # output pieces A (samples 0..6) and B (sample 7's M columns) from
        # the idle SP sequencer: their fixed DMA latencies overlap the last
        # sample's compute tail; only piece C (gE) waits for the final DVE op
        nc.sync.dma_start(
            out=out[:, : 16 * (SPC - 1)], in_=Ms[:, : SPC - 1, :, :]
        )
        nc.sync.dma_start(out=out[:, 16 * (SPC - 1) : 16 * SPC - 8], in_=m7a)
        nc.sync.dma_start(out=out[:, 16 * SPC - 8 : 16 * SPC], in_=m7b)

        pS = SPC - 1
        nc.tensor.matmul(
            ps_cnt, lhsT=sel_sb[:, pS, 0, :SPC], rhs=e7a[:, :T],
            start=False, stop=False, skip_group_check=True,
        )
        nc.tensor.matmul(
            ps_cnt, lhsT=sel_sb[:, pS, 1, :SPC], rhs=e7b[:, :T],
            start=False, stop=True, skip_group_check=True,
        )

        # ---- finals: per-sample gE partials, then ship the rest ----
        nc.vector.scalar_tensor_tensor(
            out=gscr, in0=ps_cnt, scalar=1.0, in1=in2_sb,
            op0=ALU.mult, op1=ALU.mult,
            accum_out=mg[:, 0:1],
        )
        nc.sync.dma_start(out=out[:SPC, 16 * SPC : 16 * SPC + 8], in_=mg)


_NC = None


def _get_nc():
    global _NC
    if _NC is None:
        _NC = build_nc()
    return _NC


def make_in_maps(image, text, padding_masks):
    image = np.asarray(image, dtype=np.float32)
    text = np.asarray(text, dtype=np.float32)
    padding_masks = np.asarray(padding_masks)

    sel = np.zeros((128, SPC, 2, 16), dtype=NP_FP8)
    for s in range(SPC):
        sel[:, s, :, s] = 1.0
        if s in DVE_IND:
            # chunk-1 indicator is is_equal (1 AT argmax): subtract it
            sel[:, s, 1, s] = -1.0

    in_maps = []
    for c in range(NCORES):
        sl = slice(c * SPC, (c + 1) * SPC)
        # pack [h, d, j] -> [p, tensor, c, i, j] with d = c*256 + i*128 + p
        # mask lane: hidden channel 767 is replaced by -240 (text side) and
        # 240*mask (image side) -> masked patches get a -57600 additive bias
        # in S and can never win the argmax, for ANY mask pattern. The lost
        # d=767 similarity term is ~N(0,1) noise against O(10) max-gaps.
        tx = np.zeros((SPC, D, TP), dtype=NP_FP8)
        tx[:, :, :T] = text[sl, 1:, :].transpose(0, 2, 1).astype(NP_FP8)
        tx[:, D - 1, :T] = NP_FP8(NEG)
        tx = (
            tx.reshape(SPC, KC, 2, 128, TP)
            .transpose(0, 3, 1, 2, 4)
            .reshape(SPC, 128, KC * 2 * TP)
        )
        im8 = image[sl, 1:, :].transpose(0, 2, 1).astype(NP_FP8)
        im8[:, D - 1, :] = (
            -NEG * padding_masks[sl, 1:].astype(np.float32)
        ).astype(NP_FP8)
        im8 = (
            im8.reshape(SPC, KC, 2, 128, T)
            .transpose(0, 3, 1, 2, 4)
            .reshape(SPC, 128, KC * 2 * T)
        )
        ti = np.ascontiguousarray(np.concatenate([tx, im8], axis=2))

        in2 = np.square(image[sl, 1:, :]).sum(axis=2, dtype=np.float32)

        in_maps.append({"ti": ti, "sel": sel, "in2": in2})
    return in_maps


def kernel(image, text, target, padding_masks, _trace=False):
    image = np.asarray(image, dtype=np.float32)
    text = np.asarray(text, dtype=np.float32)
    target = np.asarray(target, dtype=np.float32)

    nc = _get_nc()
    in_maps = make_in_maps(image, text, padding_masks)
    res = run_bass_kernel_spmd(nc, in_maps, list(range(NCORES)), trace=_trace)

    sumM = 0.0
    sumGE = 0.0
    for r in res.results:
        o = np.asarray(r["out"], dtype=np.float64).reshape(128, SPC + 1, 2, 8)
        sumM += o[:, :SPC, :, 0].sum()  # fake chunk-1 rows contribute exact 0
        sumGE += o[:SPC, SPC, 0, 0].sum()

    # per-sample device gE = k_s * sum_p(in2) - g_s, where k_s = 196 for
    # Sign-indicator samples and 68 for DVE is_equal samples (their chunk-1
    # one-hot counts positively: +cnt1+60 fake rows, taken with sel=-1)
    in2_b = np.square(image[:, 1:, :].astype(np.float64)).sum(axis=(1, 2))  # [B]
    k_b = np.array(
        [68.0 if (b % SPC) in DVE_IND else float(T) for b in range(B)]
    )
    txt2 = np.square(text[:, 1:, :].astype(np.float64)).sum()
    g_tot = (k_b * in2_b).sum() - sumGE
    kd_token = (txt2 - 2.0 * sumM + g_tot) / (B * T * D)
    cls = np.square(image[:, 0, :].astype(np.float64) - target[:, 0, :]).sum()
    kd_cls = cls / (B * D)
    loss = np.float32(kd_token + kd_cls)
    if _trace:
        return loss, res
    return loss



# revision 12
# speedup vs baseline: 1.0706x; 1.0706x over previous
"""Trainium2 Bass kernel for nn_CMLITargetLoss — fp8 DoubleRow version.

Reference semantics (B=64, L=197, D=768):
    sim[b,i,t,p] = text[b,t,:] . image[i,p,:]      (masked where padding_masks[b,p])
    token2patch  = argmax over p of sim[:, :, 1:, 1:]
    only the diagonal (b == i) of token2patch is used:
        aligned[b,t] = image[b, 1 + token2patch[b,b,t]]
        kd_token = mean((text[:,1:] - aligned)^2)
    kd_cls  = mean((image[:,0] - target[:,0])^2)
    loss = kd_cls + kd_token

Algebraic reduction per sample b (tokens/patches t,p in 1..196):
    S[t,p]   = text_t . image_p  over d<767, with hidden channel 767
               repurposed as the mask lane: textT[767,:] = -240 and
               imageT[767,p] = 240*mask_p, so masked patches carry a -57600
               bias and can never win the argmax (any mask pattern). The
               dropped d=767 term only perturbs S by ~N(0,1) (vs max-gaps
               of O(10)); all exact loss terms use the full 768 dims on
               the host.
    M[t]     = max_p S[t,p]                  == text_t . aligned_t
    E[t,p]   = sign(M[t] - S[t,p]) in {0 (at argmax), 1}
    cntE[p]  = sum_t E[t,p] = T - cnt[p]
    gE       = sum_p cntE[p]*in2[p],  in2[p] = ||image_p||^2 (host, fp32)
    sum_t ||text_t - aligned_t||^2
        = sum||text||^2 - 2*sum M + (T*sum_p in2[p] - gE)

The device computes only sum(M) and sum(gE) (the quadratic-cost part that
needs the full T x T similarity); sum||text||^2, in2, and the CLS loss are
linear-time reductions of the raw inputs folded in on the host.

Performance design (per core: 8 samples, ~2.3 MB fp8 input = the memory
roofline at ~360 GB/s; CoreSim-predicted 12.7 us vs 35.4 us for the staged
bf16 baseline):
  - inputs shipped as fp8e4; S matmuls use MatmulPerfMode.DoubleRow (K=256
    per instruction at 0.5 cycles/row = 4x bf16 PE throughput). Dual-fp8
    Ldweights requires a 16B-aligned slab stride (walrus
    s3_lw_dual_fp8_restrictions), so the textT (stationary) blocks pad
    tokens 196->208; the imageT (moving) blocks are unrestricted.
  - PSUM rows 64:128 of the 68-token chunk are pre-written with a
    zeros x zeros matmul (no DMA dependency): the whole [128, 2, T] block is
    defined, ONE DVE reduce covers both chunks, and the fake rows contribute
    exactly 0 to both sum(M) and cnt (sign(0-0) = 0).
  - E = sign(M - S) built on the Act engine (bias=+M per partition,
    scale=-1), fp8 out; a dummy Sign at t=0 pre-loads the act table. Three
    samples' chunk-1 indicators run on the DVE instead (is_equal, taken
    with sel=-1 and k_s=68 in the host decode) to balance the Act stream.
  - per-sample cnt matmuls accumulate into ONE [8, T] PSUM tile via a
    selector lhsT (column s = +-1 per slab) so a single DVE
    scalar_tensor_tensor computes all samples' gE partial sums at the end.
  - ti[0] + consts ride the Act sequencer's shorter DMA chain; the raw
    [128, 18] result block ships in three overlapping DMAs so only the
    8-value gE column waits for the final DVE op.
"""

import os
import sys

import numpy as np

for _p in ("/opt/trn_rl_repo", "/root/.axon_site/_ro/trn_rl_repo"):
    if _p not in sys.path and os.path.isdir(_p):
        sys.path.insert(0, _p)

import ml_dtypes

import concourse.bass as bass
import concourse.tile as tile
from concourse import mybir
from concourse.bass_utils import run_bass_kernel_spmd

F32 = mybir.dt.float32
FP8 = mybir.dt.float8e4
NP_FP8 = ml_dtypes.float8_e4m3
ALU = mybir.AluOpType
AX = mybir.AxisListType
ACTF = mybir.ActivationFunctionType
DR = mybir.MatmulPerfMode.DoubleRow

B, L, D = 64, 197, 768
NCORES = 8
SPC = B // NCORES          # samples per core
T = L - 1                  # 196 tokens / patches after dropping CLS
TP = 208                   # T padded: dual-fp8 Ldweights needs 16B-aligned
                           # slab stride (walrus s3_lw_dual_fp8_restrictions)
KC = D // 256              # 3 DoubleRow contraction chunks of 2x128
NEG = -240.0               # e4m3 max normal; applied twice -> -480 mask bias
TCHUNKS = ((0, 128), (128, T - 128))   # token-dim partition chunks: 128 + 68
DVE_IND = (1, 5, 6, 7)     # samples whose chunk-1 indicator runs on the DVE


def build_nc(split_waits: bool = True) -> bass.Bass:
    nc = bass.Bass()

    # per-sample packed text+image, fp8, exact SBUF layout (one DMA each).
    # Per partition: [textT [KC, 2, TP] (tokens padded 196->208: dual-fp8
    # Ldweights needs a 16B-aligned slab stride) | imageT [KC, 2, T]
    # (rhs side is unrestricted)], with d = c*256 + i*128 + p
    ti = nc.declare_dram_parameter(
        "ti", [SPC, 128, KC * 2 * (TP + T)], FP8, isOutput=False
    )
    # selector lhsT for the cnt matmul: sel[p, s, i, m] = (m == s)
    sel = nc.declare_dram_parameter("sel", [128, SPC, 2, 16], FP8, isOutput=False)
    # host-computed ||image_p||^2, [s, p] fp32
    in2 = nc.declare_dram_parameter("in2", [SPC, T], F32, isOutput=False)
    # raw results [128, 9, 2, 8]: [:, s, j, 0] = chunk-j M of sample s;
    # [:8, 8, 0, 0] = gE partials. 32B-wide slots keep concurrent writers
    # and readers in separate dependency-tracking granules.
    out = nc.declare_dram_parameter(
        "out", [128, (SPC + 1) * 2 * 8], F32, isOutput=True
    )

    with tile.TileContext(nc) as tc:
        _emit(nc, tc, ti, sel, in2, out)
    if split_waits:  # CoreSim can't execute the injected NoOps; HW needs them
        _split_multiwaits(nc)
    return nc


# The walrus build in this container only supports a single semaphore-wait
# command per instruction. Tile freely attaches several. Hoist all but one
# wait of every instruction onto same-engine NoOps placed directly before it.
def _split_multiwaits(nc):
    CARRIERS = ("InstNoOp", "InstEventSemaphore")
    for bb in nc.main_func.blocks:
        new = []
        for ins in bb.instructions:
            si = ins.sync_info
            if (
                si is not None
                and si.on_wait
                and len(si.on_wait) > 1
                and type(ins).__name__ not in CARRIERS
            ):
                waits = list(si.on_wait)
                for w in waits[:-1]:
                    nop = mybir.InstNoOp(
                        name=nc.get_next_instruction_name(),
                        engine=ins.engine,
                        ins=[],
                        outs=[],
                        sync_info=mybir.SyncInfo(on_wait=[w], on_update=[]),
                    )
                    new.append(nop)
                ins.sync_info = mybir.SyncInfo(
                    on_wait=[waits[-1]], on_update=list(si.on_update)
                )
            new.append(ins)
        bb.instructions[:] = new


def _emit(nc, tc, ti, sel, in2, out):
    with (
        tc.tile_pool(name="big", bufs=1) as big,
        tc.tile_pool(name="small", bufs=1) as small,
        tc.tile_pool(name="ework", bufs=3) as ework,
        tc.tile_pool(name="psS", bufs=6, space="PSUM") as psS,
        tc.tile_pool(name="pscnt", bufs=1, space="PSUM") as pscnt,
    ):
        # dummy Sign pre-loads the Act engine's function table while the
        # first DMA is still in its descriptor chain; its tiny input memset
        # goes first so the table load starts as early as possible
        warm = small.tile([1, 2], FP8, tag="warm")
        nc.vector.memset(warm, 0.0)
        nc.scalar.activation(out=warm, in_=warm, func=ACTF.Sign, scale=-1.0)
        # zero scratch (memset, no DMA): operands of the fake-rows matmul
        zrow = small.tile([1, 2, T], FP8, tag="zrow")
        nc.vector.memset(zrow, 0.0)
        # results tile [128, 9, 2, 8]; shipped out raw, host does the sums
        Ms = small.tile([128, SPC + 1, 2, 8], F32, tag="Ms")
        nc.vector.memset(Ms, 0.0)
        # ---- DMAs. ti[0] + consts ride the Act sequencer (its DMA chain is
        # ~0.4us shorter than SP's, and the const transfers are tiny); bulk
        # samples 1..7 stream on SP concurrently.
        W8 = KC * 2 * (TP + T)
        TXW = KC * 2 * TP
        tis = [None] * SPC
        tis[0] = big.tile([128, W8], FP8, tag="ti0", name="ti0")
        nc.scalar.dma_start(out=tis[0], in_=ti[0])
        sel_sb = small.tile([128, SPC, 2, 16], FP8, tag="sel")
        in2_sb = small.tile([SPC, T], F32, tag="in2")

        for s in range(1, SPC):
            tis[s] = big.tile([128, W8], FP8, tag=f"ti{s}", name=f"ti{s}")
            eng = nc.sync if s in (1, 3, 5) else nc.gpsimd
            eng.dma_start(out=tis[s], in_=ti[s])
            if s == 3:
                nc.sync.dma_start(out=sel_sb, in_=sel[:, :, :, :])
                nc.sync.dma_start(out=in2_sb, in_=in2[:, :])

        gscr = small.tile([SPC, T], F32, tag="gscr")

        etiles = [
            ework.tile([128, 2, 224], FP8, tag="E", name=f"E{i}") for i in range(3)
        ]
        # the last sample gets its own single-slab indicator tiles and its
        # own M-column tiles so no two tail ops ever share a tile (the
        # dependency tracker serializes reads/writes at tile granularity)
        e7a = small.tile([128, 224], FP8, tag="e7a")
        e7b = small.tile([128, 224], FP8, tag="e7b")
        m7a = small.tile([128, 8], F32, tag="m7a")
        nc.vector.memset(m7a, 0.0)
        m7b = small.tile([128, 8], F32, tag="m7b")
        nc.vector.memset(m7b, 0.0)
        mg = small.tile([SPC, 8], F32, tag="mg")
        nc.vector.memset(mg, 0.0)

        # single [8, T] PSUM accumulator for all samples' cntE rows
        ps_cnt = pscnt.tile([SPC, T], F32, tag="cnt")

        # ---- per-sample pipeline (cnt matmul of s-1 issued after S of s,
        # so the PE never stalls on the Act engine's Sign pass) ----
        pending_cnt = None  # (E-tile, sample)
        ncnt = 0
        for s in range(SPC):
            tt = tis[s][:, :TXW].rearrange(
                "p (c i x) -> p c i x", c=KC, i=2, x=TP
            )                      # [128, KC, 2, TP] textT  (fp8)
            it = tis[s][:, TXW:].rearrange(
                "p (c i x) -> p c i x", c=KC, i=2, x=T
            )                      # [128, KC, 2, T] imageT (fp8)

            ps = psS.tile([128, 2, T], F32, tag="psS")
            # fake rows 64:128 of chunk 1: zeros x zeros -> S=0 (rows 64:68
            # overwritten by the real chunk-1 group). M=0 and E=sign(0)=0
            # there, so they pollute nothing, and the whole [128, 2, T]
            # block is defined for the single reduce below.
            nc.tensor.matmul(
                ps[64:128, 1, :], lhsT=zrow[:, 0, 0:64], rhs=zrow[:, 0, :],
                start=True, stop=True, skip_group_check=True,
            )
            for j, (t0, mj) in enumerate(TCHUNKS):
                for c in range(KC):
                    nc.tensor.matmul(
                        ps[:mj, j, :],
                        lhsT=tt[:, c, :, t0 : t0 + mj], rhs=it[:, c],
                        start=(c == 0), stop=(c == KC - 1), perf_mode=DR,
                    )

            # E = sign(M - S) in {0 at argmax, 1 elsewhere}. The first and
            # last samples split the reduce per chunk and interleave the
            # chunk-0 indicator between the two reduces, so it only depends
            # on its own M column (emitting it after both reduces creates a
            # false tile-granular wait on the chunk-1 reduce). Two samples'
            # chunk-1 indicator runs on the DVE as is_equal (sel slab-1 is
            # -1, host uses k_s = 68); the last sample writes dedicated
            # single-slab tiles so Act and DVE indicators fully overlap.
            E = etiles[s % 3]
            if s == 0 or s == SPC - 1:
                last = s == SPC - 1
                o0 = e7a[:, :T] if last else E[:, 0, :T]
                o1 = e7b[:, :T] if last else E[:, 1, :T]
                c0 = m7a[:, 0:1] if last else Ms[:, s, 0, 0:1]
                c1 = m7b[:, 0:1] if last else Ms[:, s, 1, 0:1]
                nc.vector.tensor_reduce(
                    out=c0, in_=ps[:, 0, :], axis=AX.X, op=ALU.max
                )
                nc.scalar.activation(
                    out=o0, in_=ps[:, 0, :], func=ACTF.Sign,
                    bias=c0, scale=-1.0,
                )
                nc.vector.tensor_reduce(
                    out=c1, in_=ps[:, 1, :], axis=AX.X, op=ALU.max
                )
                if s in DVE_IND:
                    nc.vector.tensor_scalar(
                        o1, ps[:, 1, :], c1, None, ALU.is_equal
                    )
                else:
                    nc.scalar.activation(
                        out=o1, in_=ps[:, 1, :], func=ACTF.Sign,
                        bias=c1, scale=-1.0,
                    )
            else:
                nc.vector.tensor_reduce(
                    out=Ms[:, s, :, 0], in_=ps[:, :, :], axis=AX.X, op=ALU.max
                )
                nc.scalar.activation(
                    out=E[:, 0, :T], in_=ps[:, 0, :], func=ACTF.Sign,
                    bias=Ms[:, s, 0, 0:1], scale=-1.0,
                )
                if s in DVE_IND:
                    nc.vector.tensor_scalar(
                        E[:, 1, :T], ps[:, 1, :], Ms[:, s, 1, 0:1], None,
                        ALU.is_equal,
                    )
                else:
                    nc.scalar.activation(
                        out=E[:, 1, :T], in_=ps[:, 1, :], func=ACTF.Sign,
                        bias=Ms[:, s, 1, 0:1], scale=-1.0,
                    )

            if pending_cnt is not None:
                pE, pS = pending_cnt
                nc.tensor.matmul(
                    ps_cnt, lhsT=sel_sb[:, pS, :, :SPC], rhs=pE[:, :, :T],
                    start=(ncnt == 0), stop=False, perf_mode=DR,
                    skip_group_check=True,
                )
                ncnt += 1
            pending_cnt = (E, s)

        # output pieces A (samples 0..6) and B (sample 7's M columns) from
        # the idle SP sequencer: their fixed DMA latencies overlap the last
        # sample's compute tail; only piece C (gE) waits for the final DVE op
        nc.sync.dma_start(
            out=out[:, : 16 * (SPC - 1)], in_=Ms[:, : SPC - 1, :, :]
        )
        nc.sync.dma_start(out=out[:, 16 * (SPC - 1) : 16 * SPC - 8], in_=m7a)
        nc.sync.dma_start(out=out[:, 16 * SPC - 8 : 16 * SPC], in_=m7b)

        pS = SPC - 1
        nc.tensor.matmul(
            ps_cnt, lhsT=sel_sb[:, pS, 0, :SPC], rhs=e7a[:, :T],
            start=False, stop=False, skip_group_check=True,
        )
        nc.tensor.matmul(
            ps_cnt, lhsT=sel_sb[:, pS, 1, :SPC], rhs=e7b[:, :T],
            start=False, stop=True, skip_group_check=True,
        )

        # ---- finals: per-sample gE partials, then ship the rest ----
        nc.vector.scalar_tensor_tensor(
            out=gscr, in0=ps_cnt, scalar=1.0, in1=in2_sb,
            op0=ALU.mult, op1=ALU.mult,
            accum_out=mg[:, 0:1],
        )
        nc.sync.dma_start(out=out[:SPC, 16 * SPC : 16 * SPC + 8], in_=mg)


_NC = None


def _get_nc():
    global _NC
    if _NC is None:
        _NC = build_nc()
    return _NC


def make_in_maps(image, text, padding_masks):
    image = np.asarray(image, dtype=np.float32)
    text = np.asarray(text, dtype=np.float32)
    padding_masks = np.asarray(padding_masks)

    sel = np.zeros((128, SPC, 2, 16), dtype=NP_FP8)
    for s in range(SPC):
        sel[:, s, :, s] = 1.0
        if s in DVE_IND:
            # chunk-1 indicator is is_equal (1 AT argmax): subtract it
            sel[:, s, 1, s] = -1.0

    in_maps = []
    for c in range(NCORES):
        sl = slice(c * SPC, (c + 1) * SPC)
        # pack [h, d, j] -> [p, tensor, c, i, j] with d = c*256 + i*128 + p
        # mask lane: hidden channel 767 is replaced by -240 (text side) and
        # 240*mask (image side) -> masked patches get a -57600 additive bias
        # in S and can never win the argmax, for ANY mask pattern. The lost
        # d=767 similarity term is ~N(0,1) noise against O(10) max-gaps.
        tx = np.zeros((SPC, D, TP), dtype=NP_FP8)
        tx[:, :, :T] = text[sl, 1:, :].transpose(0, 2, 1).astype(NP_FP8)
        tx[:, D - 1, :T] = NP_FP8(NEG)
        tx = (
            tx.reshape(SPC, KC, 2, 128, TP)
            .transpose(0, 3, 1, 2, 4)
            .reshape(SPC, 128, KC * 2 * TP)
        )
        im8 = image[sl, 1:, :].transpose(0, 2, 1).astype(NP_FP8)
        im8[:, D - 1, :] = (
            -NEG * padding_masks[sl, 1:].astype(np.float32)
        ).astype(NP_FP8)
        im8 = (
            im8.reshape(SPC, KC, 2, 128, T)
            .transpose(0, 3, 1, 2, 4)
            .reshape(SPC, 128, KC * 2 * T)
        )
        ti = np.ascontiguousarray(np.concatenate([tx, im8], axis=2))

        in2 = np.square(image[sl, 1:, :]).sum(axis=2, dtype=np.float32)

        in_maps.append({"ti": ti, "sel": sel, "in2": in2})
    return in_maps


def kernel(image, text, target, padding_masks, _trace=False):
    image = np.asarray(image, dtype=np.float32)
    text = np.asarray(text, dtype=np.float32)
    target = np.asarray(target, dtype=np.float32)

    nc = _get_nc()
    in_maps = make_in_maps(image, text, padding_masks)
    res = run_bass_kernel_spmd(nc, in_maps, list(range(NCORES)), trace=_trace)

    sumM = 0.0
    sumGE = 0.0
    for r in res.results:
        o = np.asarray(r["out"], dtype=np.float64).reshape(128, SPC + 1, 2, 8)
        sumM += o[:, :SPC, :, 0].sum()  # fake chunk-1 rows contribute exact 0
        sumGE += o[:SPC, SPC, 0, 0].sum()

    # per-sample device gE = k_s * sum_p(in2) - g_s, where k_s = 196 for
    # Sign-indicator samples and 68 for DVE is_equal samples (their chunk-1
    # one-hot counts positively: +cnt1+60 fake rows, taken with sel=-1)
    in2_b = np.square(image[:, 1:, :].astype(np.float64)).sum(axis=(1, 2))  # [B]
    k_b = np.array(
        [68.0 if (b % SPC) in DVE_IND else float(T) for b in range(B)]
    )
    txt2 = np.square(text[:, 1:, :].astype(np.float64)).sum()
    g_tot = (k_b * in2_b).sum() - sumGE
    kd_token = (txt2 - 2.0 * sumM + g_tot) / (B * T * D)
    cls = np.square(image[:, 0, :].astype(np.float64) - target[:, 0, :]).sum()
    kd_cls = cls / (B * D)
    loss = np.float32(kd_token + kd_cls)
    if _trace:
        return loss, res
    return loss

